# revision 1
# baseline (speedup 1.0000x reference)
"""MoE expert-MLP (SwiGLU) kernel for 8 Trainium2 NeuronCores.

Strategy: expert-parallel. Each of the 8 cores owns one expert's weights.
Tokens are routed on the host: every (token, k) routing slot is dispatched to
its expert's core, padded to a fixed per-expert capacity. Each core runs a
dense [cap, D] SwiGLU MLP for its expert in fp32r (full-rate fp32 matmul
mode on the PE array) and scales rows by the routing weight. The host then
scatter-combines the K=2 per-token contributions. No collectives needed.

Per-core kernel (cap tokens, D=2048, H=1408), loops in passes of <=768
tokens so x^T, h^T stay SBUF-resident per pass while Wg/Wu stream per
h-tile. Wd stays fully resident. The first pass is smaller so the PE can
start as soon as the first x^T d-chunk lands.
  stage A: h^T[h, t] = silu(Wg @ x^T) * (Wu @ x^T)   (PSUM accumulate over D)
  stage B: y[t, d]  = (h^T)^T @ Wd^T, row-scaled by routing weight
"""

import sys
import os

sys.path.insert(0, "/opt/trn_rl_repo")

import numpy as np

T, D, H, E, K = 8192, 2048, 1408, 8, 2
P = 128
HT = H // P        # 11 h-tiles
KT = D // P        # 16 d-tiles
DC = 512           # moving-dim chunk for stage B

_built = {}


def _pass_sizes(cap):
    """Split cap into passes: first ~640 (fast startup), rest 768.
    Every size is a multiple of 128 in [256, 768]."""
    sizes = []
    rem = cap
    while rem:
        if rem <= 768:
            s = rem
        elif rem - 768 >= 256:
            s = 768
        else:
            s = rem - 256
        sizes.append(s)
        rem -= s
    assert all(s % 128 == 0 and 256 <= s <= 768 for s in sizes), sizes
    return sizes


def _tg_split(s):
    """Split a pass into <=2 moving-dim groups, each in [256, 512]."""
    if s <= 512:
        return [s]
    return [s - 384, 384]


def _build_nc(cap):
    import concourse.bass as bass  # noqa: F401
    from concourse import bacc
    import concourse.mybir as mybir
    import concourse.tile as tile

    F32 = mybir.dt.float32
    F32R = mybir.dt.float32r
    Silu = mybir.ActivationFunctionType.Silu
    Mult = mybir.AluOpType.mult

    sizes = _pass_sizes(cap)

    nc = bacc.Bacc("TRN2", target_bir_lowering=False, debug=False)
    xT = nc.declare_dram_parameter("xT", [D, cap], F32R, isOutput=False)
    wg = nc.declare_dram_parameter("wg", [HT, P, KT * P], F32R, isOutput=False)
    wu = nc.declare_dram_parameter("wu", [HT, P, KT * P], F32R, isOutput=False)
    wd = nc.declare_dram_parameter("wd", [H, D], F32R, isOutput=False)
    wt = nc.declare_dram_parameter("wt", [cap], F32, isOutput=False)
    out = nc.declare_dram_parameter("out", [cap, D], F32, isOutput=True)

    with tile.TileContext(nc) as tc:
        with (
            tc.tile_pool(name="sbuf", bufs=1) as pool,
            tc.tile_pool(name="psum", bufs=1, space="PSUM") as pp,
        ):
            wd_ts = [None] * HT
            wt_t = None
            t0 = 0
            for pi, TC in enumerate(sizes):
                tgs = _tg_split(TC)
                # x^T for this pass, one tile per d-tile so the first
                # matmuls only wait on the first chunk's DMA
                xt_ts = []
                for dti in range(KT):
                    xt_1 = pool.tile([P, TC], F32R, tag=f"xt{dti}", bufs=1,
                                     name=f"xt{dti}")
                    nc.gpsimd.dma_start(
                        xt_1[:], xT[dti * P : (dti + 1) * P, t0 : t0 + TC]
                    )
                    xt_ts.append(xt_1)
                # h^T for this pass
                h_t = pool.tile([P, HT, TC], F32R, tag="ht", bufs=1)

                # ---- stage A: h^T = silu(g^T) * u^T ----
                for ht in range(HT):
                    wg_t = pool.tile([P, KT * P], F32R, tag="wgu", bufs=3)
                    nc.sync.dma_start(wg_t[:], wg[ht, :, :])
                    wu_t = pool.tile([P, KT * P], F32R, tag="wgu", bufs=3)
                    nc.sync.dma_start(wu_t[:], wu[ht, :, :])
                    if pi == 0:
                        # stream Wd piecewise during pass 0 so it doesn't
                        # contend with the Wg/Wu stream on one DGE FIFO
                        wdc = pool.tile([P, D], F32R, tag=f"wd{ht}", bufs=1,
                                        name=f"wdc{ht}")
                        nc.gpsimd.dma_start(
                            wdc[:], wd[ht * P : (ht + 1) * P, :]
                        )
                        wd_ts[ht] = wdc

                    psg = [pp.tile([P, g], F32, tag=f"g{i}", bufs=1,
                                   name=f"psg{i}") for i, g in enumerate(tgs)]
                    psu = [pp.tile([P, g], F32, tag=f"u{i}", bufs=1,
                                   name=f"psu{i}") for i, g in enumerate(tgs)]
                    off = [0, tgs[0]]
                    for d in range(KT):
                        lhs = wg_t[:, d * P : (d + 1) * P]
                        for tg, g in enumerate(tgs):
                            nc.tensor.matmul(
                                psg[tg][:],
                                lhs,
                                xt_ts[d][:, off[tg] : off[tg] + g],
                                start=(d == 0),
                                stop=(d == KT - 1),
                            )
                    silu_ts = []
                    for tg, g in enumerate(tgs):
                        st = pool.tile([P, g], F32, tag="silu", bufs=2,
                                       name="st")
                        nc.scalar.activation(st[:], psg[tg][:], Silu)
                        silu_ts.append(st)
                    for d in range(KT):
                        lhs = wu_t[:, d * P : (d + 1) * P]
                        for tg, g in enumerate(tgs):
                            nc.tensor.matmul(
                                psu[tg][:],
                                lhs,
                                xt_ts[d][:, off[tg] : off[tg] + g],
                                start=(d == 0),
                                stop=(d == KT - 1),
                            )
                    for tg, g in enumerate(tgs):
                        nc.vector.tensor_tensor(
                            h_t[:, ht, off[tg] : off[tg] + g],
                            silu_ts[tg][:],
                            psu[tg][:],
                            op=Mult,
                        )

                if pi == 0:
                    wt_t = pool.tile([P, cap // P], F32, tag="wt", bufs=1)
                    nc.gpsimd.dma_start(
                        wt_t[:], wt.rearrange("(n p) -> p n", p=P)
                    )

                # ---- stage B: y = h @ Wd^T, scaled by routing weight ----
                for ts_ in range(TC // P):
                    psy = [pp.tile([P, DC], F32, tag=f"y{i}", bufs=1,
                                   name=f"psy{i}") for i in range(4)]
                    for ht in range(HT):
                        lhs = h_t[:, ht, ts_ * P : (ts_ + 1) * P]
                        for dc in range(4):
                            nc.tensor.matmul(
                                psy[dc][:],
                                lhs,
                                wd_ts[ht][:, dc * DC : (dc + 1) * DC],
                                start=(ht == 0),
                                stop=(ht == HT - 1),
                            )
                    col = t0 // P + ts_
                    for half in range(2):
                        y_t = pool.tile([P, D // 2], F32, tag="yout", bufs=2,
                                        name="y_t")
                        for j in range(2):
                            dc = half * 2 + j
                            nc.vector.tensor_scalar_mul(
                                y_t[:, j * DC : (j + 1) * DC],
                                psy[dc][:],
                                wt_t[:, col : col + 1],
                            )
                        nc.sync.dma_start(
                            out[
                                t0 + ts_ * P : t0 + (ts_ + 1) * P,
                                half * (D // 2) : (half + 1) * (D // 2),
                            ],
                            y_t[:],
                        )
                t0 += TC

    nc.finalize()
    return nc


def _get_nc(cap):
    if cap not in _built:
        _built[cap] = _build_nc(cap)
    return _built[cap]


def kernel(x, weights, Wg, Wu, Wd, indices, seq_len=None, **_unused):
    from concourse.bass_utils import run_bass_kernel_spmd

    x = np.asarray(x, dtype=np.float32)
    weights = np.asarray(weights, dtype=np.float32)
    Wg = np.asarray(Wg, dtype=np.float32)
    Wu = np.asarray(Wu, dtype=np.float32)
    Wd = np.asarray(Wd, dtype=np.float32)
    indices = np.asarray(indices)

    t, d = x.shape
    e, h, _ = Wg.shape
    k = indices.shape[1]

    # ---- host-side routing (dispatch) ----
    flat_e = indices.reshape(-1).astype(np.int64)
    flat_w = weights.reshape(-1)
    flat_t = np.repeat(np.arange(t, dtype=np.int64), k)
    order = np.argsort(flat_e, kind="stable")
    counts = np.bincount(flat_e, minlength=e)
    starts = np.zeros(e + 1, dtype=np.int64)
    starts[1:] = np.cumsum(counts)
    cap = int(-(-max(int(counts.max()), 512) // P) * P)

    tok_sorted = flat_t[order]
    w_sorted = flat_w[order]

    in_maps = []
    for ei in range(e):
        n = int(counts[ei])
        toks = tok_sorted[starts[ei] : starts[ei] + n]
        xe = np.zeros((cap, d), dtype=np.float32)
        xe[:n] = x[toks]
        wvec = np.zeros(cap, dtype=np.float32)
        wvec[:n] = w_sorted[starts[ei] : starts[ei] + n]
        # pack Wg/Wu so each h-tile block is one contiguous [128, 2048] DMA:
        # block[ht][p][k*128+hh] = Wg[e].T[k*128+p, ht*128+hh]
        WgT = Wg[ei].T  # [D, H]
        WuT = Wu[ei].T
        wg_lin = np.ascontiguousarray(
            WgT.reshape(KT, P, HT, P).transpose(2, 1, 0, 3).reshape(HT, P, KT * P)
        )
        wu_lin = np.ascontiguousarray(
            WuT.reshape(KT, P, HT, P).transpose(2, 1, 0, 3).reshape(HT, P, KT * P)
        )
        wdT = np.ascontiguousarray(Wd[ei].T)  # [H, D]
        in_maps.append(
            {
                "xT": np.ascontiguousarray(xe.T),
                "wg": wg_lin,
                "wu": wu_lin,
                "wd": wdT,
                "wt": wvec,
            }
        )

    nc = _get_nc(cap)
    trace = bool(int(os.environ.get("KERNEL_TRACE", "0")))
    res = run_bass_kernel_spmd(
        nc, in_maps, core_ids=list(range(e)), trace=trace
    )
    if trace:
        kernel.last_exec_time_ns = res.exec_time_ns
        kernel.last_results = res

    # ---- host-side combine ----
    allres = np.concatenate(
        [res.results[ei]["out"][: counts[ei]] for ei in range(e)], axis=0
    )
    inv = np.empty(t * k, dtype=np.int64)
    inv[order] = np.arange(t * k, dtype=np.int64)
    y = allres[inv].reshape(t, k, d).sum(axis=1, dtype=np.float32)
    return y



# revision 6
# speedup vs baseline: 1.1575x; 1.1575x over previous
"""MoE expert-MLP (SwiGLU) kernel for 8 Trainium2 NeuronCores.

Strategy: expert-parallel, one expert per core. Host-side routing dedups
duplicate (token, expert) slots (K=2 can pick the same expert twice; the
routing weights then just add), which drops the per-expert max count under
2048 and saves a full 128-token tile of padding. Each core runs a dense
[cap, D] SwiGLU MLP in bf16 (tolerance is 2e-2; bf16 lands ~5e-3) which
halves DMA traffic and SBUF footprint vs fp32r at the same PE rate.

Per-core kernel, single pass (cap <= 2048): x^T, h^T and Wd stay fully
SBUF-resident; Wg/Wu stream once per h-tile. PSUM is treated as 8 uniform
[128, 512] f32 banks: stage A uses 4 gate + 4 up accumulators; stage B
ping-pongs its 4 output accumulators between the two stage-A sets so
back-to-back t-tiles never hit a write-after-read stall.
  stage A: h^T[h, t] = silu(Wg @ x^T) * (Wu @ x^T)   (PSUM accumulate over D)
  stage B: y[t, d]  = (h^T)^T @ Wd^T, row-scaled by routing weight
"""

import sys
import os

sys.path.insert(0, "/opt/trn_rl_repo")

import numpy as np

T, D, H, E, K = 8192, 2048, 1408, 8, 2
P = 128
HT = H // P        # 11 h-tiles
KT = D // P        # 16 d-tiles
G = 512            # PSUM group width (one bank of f32)

_built = {}


def _pass_sizes(cap):
    """Split cap into passes of <=2048 tokens (multiples of 128).
    Expected input fits in a single pass of 2048."""
    sizes = []
    rem = cap
    while rem:
        s = min(rem, 2048)
        if rem - s == 128:  # avoid a tiny trailing pass
            s -= 128
        sizes.append(s)
        rem -= s
    assert all(s % 128 == 0 for s in sizes), sizes
    return sizes


def _groups(tc):
    """Split a pass into 512-wide PSUM groups (last may be smaller)."""
    return [(o, min(G, tc - o)) for o in range(0, tc, G)]


def _build_nc(cap):
    import concourse.bass as bass  # noqa: F401
    from concourse import bacc
    import concourse.mybir as mybir
    import concourse.tile as tile

    F32 = mybir.dt.float32
    BF16 = mybir.dt.bfloat16
    Silu = mybir.ActivationFunctionType.Silu
    Mult = mybir.AluOpType.mult

    sizes = _pass_sizes(cap)
    maxtc = max(sizes)

    nc = bacc.Bacc("TRN2", target_bir_lowering=False, debug=False)
    xT = nc.declare_dram_parameter("xT", [D, cap], BF16, isOutput=False)
    wg = nc.declare_dram_parameter("wg", [HT, P, KT * P], BF16, isOutput=False)
    wu = nc.declare_dram_parameter("wu", [HT, P, KT * P], BF16, isOutput=False)
    wd = nc.declare_dram_parameter("wd", [H, D], BF16, isOutput=False)
    wt = nc.declare_dram_parameter("wt", [cap], F32, isOutput=False)
    out = nc.declare_dram_parameter("out", [cap, D], BF16, isOutput=True)

    with tile.TileContext(nc) as tc:
        with (
            tc.tile_pool(name="sbuf", bufs=1) as pool,
            tc.tile_pool(name="psum", bufs=1, space="PSUM") as pp,
        ):
            wd_ts = [None] * HT
            wt_t = None
            t0 = 0
            for pi, TC in enumerate(sizes):
                grps = _groups(TC)
                ng = len(grps)
                # x^T for this pass, one tile per d-tile, spread over two
                # DMA queues so the PE isn't gated by a single x stream
                xt_ts = []
                for dti in range(KT):
                    xt_1 = pool.tile([P, maxtc], BF16, tag=f"xt{dti}", bufs=1,
                                     name=f"xt{dti}")
                    eng = nc.gpsimd if dti % 2 == 0 else nc.scalar
                    eng.dma_start(
                        xt_1[:, :TC], xT[dti * P : (dti + 1) * P, t0 : t0 + TC]
                    )
                    xt_ts.append(xt_1)
                # h^T for this pass
                h_t = pool.tile([P, HT, maxtc], BF16, tag="ht", bufs=1)

                # ---- stage A: h^T = silu(g^T) * u^T ----
                for ht in range(HT):
                    wg_t = pool.tile([P, KT * P], BF16, tag="wgu", bufs=4)
                    nc.sync.dma_start(wg_t[:], wg[ht, :, :])
                    wu_t = pool.tile([P, KT * P], BF16, tag="wgu", bufs=4)
                    nc.sync.dma_start(wu_t[:], wu[ht, :, :])
                    if pi == 0:
                        # Wd is resident for the whole kernel; stream it on
                        # the sync queue during the first stage A
                        wdc = pool.tile([P, D], BF16, tag=f"wd{ht}", bufs=1,
                                        name=f"wdc{ht}")
                        nc.sync.dma_start(
                            wdc[:], wd[ht * P : (ht + 1) * P, :]
                        )
                        wd_ts[ht] = wdc

                    psg = [pp.tile([P, G], F32, tag=f"g{i}", bufs=1,
                                   name=f"psg{i}") for i in range(ng)]
                    psu = [pp.tile([P, G], F32, tag=f"u{i}", bufs=1,
                                   name=f"psu{i}") for i in range(ng)]
                    for d in range(KT):
                        lhs = wg_t[:, d * P : (d + 1) * P]
                        for gi, (off, g) in enumerate(grps):
                            nc.tensor.matmul(
                                psg[gi][:, :g],
                                lhs,
                                xt_ts[d][:, off : off + g],
                                start=(d == 0),
                                stop=(d == KT - 1),
                            )
                    silu_ts = []
                    for gi, (off, g) in enumerate(grps):
                        st = pool.tile([P, G], F32, tag=f"silu{gi}", bufs=2,
                                       name="st")
                        nc.scalar.activation(st[:, :g], psg[gi][:, :g], Silu)
                        silu_ts.append(st)
                    for d in range(KT):
                        lhs = wu_t[:, d * P : (d + 1) * P]
                        for gi, (off, g) in enumerate(grps):
                            nc.tensor.matmul(
                                psu[gi][:, :g],
                                lhs,
                                xt_ts[d][:, off : off + g],
                                start=(d == 0),
                                stop=(d == KT - 1),
                            )
                    for gi, (off, g) in enumerate(grps):
                        nc.vector.tensor_tensor(
                            h_t[:, ht, off : off + g],
                            silu_ts[gi][:, :g],
                            psu[gi][:, :g],
                            op=Mult,
                        )

                if pi == 0:
                    wt_t = pool.tile([P, cap // P], F32, tag="wt", bufs=1)
                    nc.gpsimd.dma_start(
                        wt_t[:], wt.rearrange("(n p) -> p n", p=P)
                    )

                # ---- stage B: y = h @ Wd^T, scaled by routing weight ----
                for ts_ in range(TC // P):
                    # ping-pong PSUM between the stage-A gate and up banks
                    # so consecutive t-tiles never wait on a WAR hazard
                    pfx = "u" if ts_ % 2 == 0 else "g"
                    psy = [pp.tile([P, G], F32, tag=f"{pfx}{i}",
                                   name=f"psy{i}") for i in range(4)]
                    y_t = pool.tile([P, D], BF16, tag="yout", bufs=2,
                                    name="y_t")
                    col = t0 // P + ts_
                    lhs_base = ts_ * P
                    for dc in range(4):
                        for ht in range(HT):
                            nc.tensor.matmul(
                                psy[dc][:],
                                h_t[:, ht, lhs_base : lhs_base + P],
                                wd_ts[ht][:, dc * G : (dc + 1) * G],
                                start=(ht == 0),
                                stop=(ht == HT - 1),
                            )
                        nc.vector.tensor_scalar_mul(
                            y_t[:, dc * G : (dc + 1) * G],
                            psy[dc][:],
                            wt_t[:, col : col + 1],
                        )
                    nc.scalar.dma_start(
                        out[t0 + ts_ * P : t0 + (ts_ + 1) * P, :],
                        y_t[:],
                    )
                t0 += TC

    nc.finalize()
    return nc


def _get_nc(cap):
    if cap not in _built:
        _built[cap] = _build_nc(cap)
    return _built[cap]


def kernel(x, weights, Wg, Wu, Wd, indices, seq_len=None, **_unused):
    from concourse.bass_utils import run_bass_kernel_spmd
    import ml_dtypes

    bf16 = ml_dtypes.bfloat16

    x = np.asarray(x, dtype=np.float32)
    weights = np.asarray(weights, dtype=np.float32)
    Wg = np.asarray(Wg, dtype=np.float32)
    Wu = np.asarray(Wu, dtype=np.float32)
    Wd = np.asarray(Wd, dtype=np.float32)
    indices = np.asarray(indices).astype(np.int64)

    t, d = x.shape
    e, h, _ = Wg.shape
    k = indices.shape[1]

    # ---- host-side routing (dispatch), merging duplicate (token, expert)
    # slots so their routing weights add and each pair is computed once ----
    flat_e = indices.reshape(-1)
    flat_t = np.repeat(np.arange(t, dtype=np.int64), k)
    flat_w = weights.reshape(-1)
    key = flat_e * t + flat_t                       # sort by (expert, token)
    order = np.argsort(key, kind="stable")
    ks = key[order]
    ws = flat_w[order]
    uniq = np.empty(len(ks), dtype=bool)
    uniq[0] = True
    uniq[1:] = ks[1:] != ks[:-1]
    seg = np.cumsum(uniq) - 1
    w_u = np.bincount(seg, weights=ws).astype(np.float32)  # summed weights
    k_u = ks[uniq]
    e_u = k_u // t
    tok_u = k_u % t
    counts = np.bincount(e_u, minlength=e)
    starts = np.zeros(e + 1, dtype=np.int64)
    starts[1:] = np.cumsum(counts)
    cap = int(-(-max(int(counts.max()), 512) // P) * P)

    in_maps = []
    for ei in range(e):
        n = int(counts[ei])
        toks = tok_u[starts[ei] : starts[ei] + n]
        xe = np.zeros((cap, d), dtype=np.float32)
        xe[:n] = x[toks]
        wvec = np.zeros(cap, dtype=np.float32)
        wvec[:n] = w_u[starts[ei] : starts[ei] + n]
        # pack Wg/Wu so each h-tile block is one contiguous [128, 2048] DMA:
        # block[ht][p][k*128+hh] = Wg[e].T[k*128+p, ht*128+hh]
        WgT = Wg[ei].T  # [D, H]
        WuT = Wu[ei].T
        wg_lin = np.ascontiguousarray(
            WgT.reshape(KT, P, HT, P).transpose(2, 1, 0, 3).reshape(HT, P, KT * P)
        ).astype(bf16)
        wu_lin = np.ascontiguousarray(
            WuT.reshape(KT, P, HT, P).transpose(2, 1, 0, 3).reshape(HT, P, KT * P)
        ).astype(bf16)
        wdT = np.ascontiguousarray(Wd[ei].T).astype(bf16)  # [H, D]
        in_maps.append(
            {
                "xT": np.ascontiguousarray(xe.T).astype(bf16),
                "wg": wg_lin,
                "wu": wu_lin,
                "wd": wdT,
                "wt": wvec,
            }
        )

    nc = _get_nc(cap)
    trace = bool(int(os.environ.get("KERNEL_TRACE", "0")))
    res = run_bass_kernel_spmd(
        nc, in_maps, core_ids=list(range(e)), trace=trace
    )
    if trace:
        kernel.last_exec_time_ns = res.exec_time_ns
        kernel.last_results = res

    # ---- host-side combine ----
    allres = np.concatenate(
        [np.asarray(res.results[ei]["out"][: counts[ei]]) for ei in range(e)],
        axis=0,
    ).astype(np.float32)
    allres = np.vstack([allres, np.zeros((1, d), np.float32)])
    # map each original (token, k) slot to its deduped row; the second slot
    # of an intra-token duplicate points at the trailing zeros row
    pos = np.searchsorted(k_u, key).reshape(t, k)
    dup = indices[:, 0] == indices[:, 1]
    pos[dup, 1] = len(k_u)
    y = allres[pos[:, 0]] + allres[pos[:, 1]]
    return y


# revision 9
# speedup vs baseline: 1.1624x; 1.0042x over previous
"""MoE expert-MLP (SwiGLU) kernel for 8 Trainium2 NeuronCores.

Strategy: expert-parallel, one expert per core. Host-side routing dedups
duplicate (token, expert) slots (K=2 can pick the same expert twice; the
routing weights then just add), which drops the per-expert max count under
2048 and saves a full 128-token tile of padding. Each core runs a dense
[cap, D] SwiGLU MLP in bf16 (tolerance is 2e-2; bf16 lands ~5e-3) which
halves DMA traffic and SBUF footprint vs fp32r at the same PE rate.

Per-core kernel, single pass (cap <= 2048): x^T, h^T and Wd stay fully
SBUF-resident; Wg/Wu stream once per h-tile. PSUM is treated as 8 uniform
[128, 512] f32 banks: stage A uses 4 gate + 4 up accumulators; stage B
ping-pongs its 4 output accumulators between the two stage-A sets so
back-to-back t-tiles never hit a write-after-read stall.
  stage A: h^T[h, t] = silu(Wg @ x^T) * (Wu @ x^T)   (PSUM accumulate over D)
  stage B: y[t, d]  = (h^T)^T @ Wd^T, row-scaled by routing weight
"""

import sys
import os

sys.path.insert(0, "/opt/trn_rl_repo")

import numpy as np

T, D, H, E, K = 8192, 2048, 1408, 8, 2
P = 128
HT = H // P        # 11 h-tiles
KT = D // P        # 16 d-tiles
G = 512            # PSUM group width (one bank of f32)

_built = {}


def _pass_sizes(cap):
    """Split cap into passes of <=2048 tokens (multiples of 128).
    Expected input fits in a single pass of 2048."""
    sizes = []
    rem = cap
    while rem:
        s = min(rem, 2048)
        if rem - s == 128:  # avoid a tiny trailing pass
            s -= 128
        sizes.append(s)
        rem -= s
    assert all(s % 128 == 0 for s in sizes), sizes
    return sizes


def _groups(tc):
    """Split a pass into 512-wide PSUM groups (last may be smaller)."""
    return [(o, min(G, tc - o)) for o in range(0, tc, G)]


def _build_nc(cap):
    import concourse.bass as bass  # noqa: F401
    from concourse import bacc
    import concourse.mybir as mybir
    import concourse.tile as tile

    F32 = mybir.dt.float32
    BF16 = mybir.dt.bfloat16
    Silu = mybir.ActivationFunctionType.Silu
    Mult = mybir.AluOpType.mult

    sizes = _pass_sizes(cap)
    maxtc = max(sizes)

    nc = bacc.Bacc("TRN2", target_bir_lowering=False, debug=False)
    xT = nc.declare_dram_parameter("xT", [D, cap], BF16, isOutput=False)
    wg = nc.declare_dram_parameter("wg", [HT, P, KT * P], BF16, isOutput=False)
    wu = nc.declare_dram_parameter("wu", [HT, P, KT * P], BF16, isOutput=False)
    wd = nc.declare_dram_parameter("wd", [H, D], BF16, isOutput=False)
    wt = nc.declare_dram_parameter("wt", [cap], F32, isOutput=False)
    out = nc.declare_dram_parameter("out", [cap, D], BF16, isOutput=True)

    with tile.TileContext(nc) as tc:
        with (
            tc.tile_pool(name="sbuf", bufs=1) as pool,
            tc.tile_pool(name="psum", bufs=1, space="PSUM") as pp,
        ):
            wd_ts = [None] * HT
            wt_t = None
            t0 = 0
            for pi, TC in enumerate(sizes):
                grps = _groups(TC)
                ng = len(grps)
                # x^T for this pass, one tile per d-tile. Each tile is
                # filled by two half-DMAs on the two spare queues (gpsimd +
                # scalar), alternating so both queues deliver d-tiles in
                # order and the PE is never gated by a single x stream.
                xt_ts = []
                half = (TC + 1) // 2 if TC > P else TC
                for dti in range(KT):
                    xt_1 = pool.tile([P, maxtc], BF16, tag=f"xt{dti}", bufs=1,
                                     name=f"xt{dti}")
                    q0, q1 = ((nc.gpsimd, nc.scalar) if dti % 2 == 0
                              else (nc.scalar, nc.gpsimd))
                    q0.dma_start(
                        xt_1[:, :half],
                        xT[dti * P : (dti + 1) * P, t0 : t0 + half],
                    )
                    if half < TC:
                        q1.dma_start(
                            xt_1[:, half:TC],
                            xT[dti * P : (dti + 1) * P, t0 + half : t0 + TC],
                        )
                    xt_ts.append(xt_1)
                # h^T for this pass
                h_t = pool.tile([P, HT, maxtc], BF16, tag="ht", bufs=1)

                # ---- stage A: h^T = silu(g^T) * u^T ----
                # Wg/Wu stream as [128, 1024] half-tiles so the first
                # matmul only waits for an 8-d-tile half, not a full row
                HW = KT * P // 2
                for ht in range(HT):
                    wg_h = []
                    wu_h = []
                    for hf in range(2):
                        w1 = pool.tile([P, HW], BF16, tag="wgu", bufs=8)
                        nc.sync.dma_start(w1[:], wg[ht, :, hf * HW : (hf + 1) * HW])
                        wg_h.append(w1)
                    for hf in range(2):
                        w1 = pool.tile([P, HW], BF16, tag="wgu", bufs=8)
                        nc.sync.dma_start(w1[:], wu[ht, :, hf * HW : (hf + 1) * HW])
                        wu_h.append(w1)

                    psg = [pp.tile([P, G], F32, tag=f"g{i}", bufs=1,
                                   name=f"psg{i}") for i in range(ng)]
                    psu = [pp.tile([P, G], F32, tag=f"u{i}", bufs=1,
                                   name=f"psu{i}") for i in range(ng)]
                    for d in range(KT):
                        lhs = wg_h[d // 8][:, (d % 8) * P : (d % 8 + 1) * P]
                        for gi, (off, g) in enumerate(grps):
                            nc.tensor.matmul(
                                psg[gi][:, :g],
                                lhs,
                                xt_ts[d][:, off : off + g],
                                start=(d == 0),
                                stop=(d == KT - 1),
                            )
                    silu_ts = []
                    for gi, (off, g) in enumerate(grps):
                        st = pool.tile([P, G], F32, tag=f"silu{gi}", bufs=2,
                                       name="st")
                        nc.scalar.activation(st[:, :g], psg[gi][:, :g], Silu)
                        silu_ts.append(st)
                    for d in range(KT):
                        lhs = wu_h[d // 8][:, (d % 8) * P : (d % 8 + 1) * P]
                        for gi, (off, g) in enumerate(grps):
                            nc.tensor.matmul(
                                psu[gi][:, :g],
                                lhs,
                                xt_ts[d][:, off : off + g],
                                start=(d == 0),
                                stop=(d == KT - 1),
                            )
                    for gi, (off, g) in enumerate(grps):
                        nc.vector.tensor_tensor(
                            h_t[:, ht, off : off + g],
                            silu_ts[gi][:, :g],
                            psu[gi][:, :g],
                            op=Mult,
                        )

                if pi == 0:
                    # Wd is resident for the whole kernel; stream it on the
                    # sync queue after the Wg/Wu stream so it stays out of
                    # the bandwidth-critical startup window
                    for ht in range(HT):
                        wdc = pool.tile([P, D], BF16, tag=f"wd{ht}", bufs=1,
                                        name=f"wdc{ht}")
                        nc.sync.dma_start(
                            wdc[:], wd[ht * P : (ht + 1) * P, :]
                        )
                        wd_ts[ht] = wdc

                if pi == 0:
                    wt_t = pool.tile([P, cap // P], F32, tag="wt", bufs=1)
                    nc.gpsimd.dma_start(
                        wt_t[:], wt.rearrange("(n p) -> p n", p=P)
                    )

                # ---- stage B: y = h @ Wd^T, scaled by routing weight ----
                for ts_ in range(TC // P):
                    # ping-pong PSUM between the stage-A gate and up banks
                    # so consecutive t-tiles never wait on a WAR hazard
                    pfx = "u" if ts_ % 2 == 0 else "g"
                    psy = [pp.tile([P, G], F32, tag=f"{pfx}{i}",
                                   name=f"psy{i}") for i in range(4)]
                    y_t = pool.tile([P, D], BF16, tag="yout", bufs=2,
                                    name="y_t")
                    col = t0 // P + ts_
                    lhs_base = ts_ * P
                    for dc in range(4):
                        for ht in range(HT):
                            nc.tensor.matmul(
                                psy[dc][:],
                                h_t[:, ht, lhs_base : lhs_base + P],
                                wd_ts[ht][:, dc * G : (dc + 1) * G],
                                start=(ht == 0),
                                stop=(ht == HT - 1),
                            )
                        nc.vector.tensor_scalar_mul(
                            y_t[:, dc * G : (dc + 1) * G],
                            psy[dc][:],
                            wt_t[:, col : col + 1],
                        )
                        nc.scalar.dma_start(
                            out[
                                t0 + ts_ * P : t0 + (ts_ + 1) * P,
                                dc * G : (dc + 1) * G,
                            ],
                            y_t[:, dc * G : (dc + 1) * G],
                        )
                t0 += TC

    nc.finalize()
    return nc


def _get_nc(cap):
    if cap not in _built:
        _built[cap] = _build_nc(cap)
    return _built[cap]


def kernel(x, weights, Wg, Wu, Wd, indices, seq_len=None, **_unused):
    from concourse.bass_utils import run_bass_kernel_spmd
    import ml_dtypes

    bf16 = ml_dtypes.bfloat16

    x = np.asarray(x, dtype=np.float32)
    weights = np.asarray(weights, dtype=np.float32)
    Wg = np.asarray(Wg, dtype=np.float32)
    Wu = np.asarray(Wu, dtype=np.float32)
    Wd = np.asarray(Wd, dtype=np.float32)
    indices = np.asarray(indices).astype(np.int64)

    t, d = x.shape
    e, h, _ = Wg.shape
    k = indices.shape[1]

    # ---- host-side routing (dispatch), merging duplicate (token, expert)
    # slots so their routing weights add and each pair is computed once ----
    flat_e = indices.reshape(-1)
    flat_t = np.repeat(np.arange(t, dtype=np.int64), k)
    flat_w = weights.reshape(-1)
    key = flat_e * t + flat_t                       # sort by (expert, token)
    order = np.argsort(key, kind="stable")
    ks = key[order]
    ws = flat_w[order]
    uniq = np.empty(len(ks), dtype=bool)
    uniq[0] = True
    uniq[1:] = ks[1:] != ks[:-1]
    seg = np.cumsum(uniq) - 1
    w_u = np.bincount(seg, weights=ws).astype(np.float32)  # summed weights
    k_u = ks[uniq]
    e_u = k_u // t
    tok_u = k_u % t
    counts = np.bincount(e_u, minlength=e)
    starts = np.zeros(e + 1, dtype=np.int64)
    starts[1:] = np.cumsum(counts)
    cap = int(-(-max(int(counts.max()), 512) // P) * P)

    in_maps = []
    for ei in range(e):
        n = int(counts[ei])
        toks = tok_u[starts[ei] : starts[ei] + n]
        xe = np.zeros((cap, d), dtype=np.float32)
        xe[:n] = x[toks]
        wvec = np.zeros(cap, dtype=np.float32)
        wvec[:n] = w_u[starts[ei] : starts[ei] + n]
        # pack Wg/Wu so each h-tile block is one contiguous [128, 2048] DMA:
        # block[ht][p][k*128+hh] = Wg[e].T[k*128+p, ht*128+hh]
        WgT = Wg[ei].T  # [D, H]
        WuT = Wu[ei].T
        wg_lin = np.ascontiguousarray(
            WgT.reshape(KT, P, HT, P).transpose(2, 1, 0, 3).reshape(HT, P, KT * P)
        ).astype(bf16)
        wu_lin = np.ascontiguousarray(
            WuT.reshape(KT, P, HT, P).transpose(2, 1, 0, 3).reshape(HT, P, KT * P)
        ).astype(bf16)
        wdT = np.ascontiguousarray(Wd[ei].T).astype(bf16)  # [H, D]
        in_maps.append(
            {
                "xT": np.ascontiguousarray(xe.T).astype(bf16),
                "wg": wg_lin,
                "wu": wu_lin,
                "wd": wdT,
                "wt": wvec,
            }
        )

    nc = _get_nc(cap)
    trace = bool(int(os.environ.get("KERNEL_TRACE", "0")))
    res = run_bass_kernel_spmd(
        nc, in_maps, core_ids=list(range(e)), trace=trace
    )
    if trace:
        kernel.last_exec_time_ns = res.exec_time_ns
        kernel.last_results = res

    # ---- host-side combine ----
    allres = np.concatenate(
        [np.asarray(res.results[ei]["out"][: counts[ei]]) for ei in range(e)],
        axis=0,
    ).astype(np.float32)
    allres = np.vstack([allres, np.zeros((1, d), np.float32)])
    # map each original (token, k) slot to its deduped row; the second slot
    # of an intra-token duplicate points at the trailing zeros row
    pos = np.searchsorted(k_u, key).reshape(t, k)
    dup = indices[:, 0] == indices[:, 1]
    pos[dup, 1] = len(k_u)
    y = allres[pos[:, 0]] + allres[pos[:, 1]]
    return y


# revision 11
# speedup vs baseline: 1.1643x; 1.0017x over previous
"""MoE expert-MLP (SwiGLU) kernel for 8 Trainium2 NeuronCores.

Strategy: expert-parallel, one expert per core. Host-side routing dedups
duplicate (token, expert) slots (K=2 can pick the same expert twice; the
routing weights then just add), which drops the per-expert max count under
2048 and saves a full 128-token tile of padding. Each core runs a dense
[cap, D] SwiGLU MLP in bf16 (tolerance is 2e-2; bf16 lands ~5e-3) which
halves DMA traffic and SBUF footprint vs fp32r at the same PE rate.

Per-core kernel, single pass (cap <= 2048): x^T, h^T and Wd stay fully
SBUF-resident; Wg/Wu stream once per h-tile. PSUM is treated as 8 uniform
[128, 512] f32 banks: stage A uses 4 gate + 4 up accumulators; stage B
ping-pongs its 4 output accumulators between the two stage-A sets so
back-to-back t-tiles never hit a write-after-read stall.
  stage A: h^T[h, t] = silu(Wg @ x^T) * (Wu @ x^T)   (PSUM accumulate over D)
  stage B: y[t, d]  = (h^T)^T @ Wd^T, row-scaled by routing weight
"""

import sys
import os

sys.path.insert(0, "/opt/trn_rl_repo")

import numpy as np

T, D, H, E, K = 8192, 2048, 1408, 8, 2
P = 128
HT = H // P        # 11 h-tiles
KT = D // P        # 16 d-tiles
G = 512            # PSUM group width (one bank of f32)

_built = {}


def _pass_sizes(cap):
    """Split cap into passes of <=2048 tokens (multiples of 128).
    Expected input fits in a single pass of 2048."""
    sizes = []
    rem = cap
    while rem:
        s = min(rem, 2048)
        if rem - s == 128:  # avoid a tiny trailing pass
            s -= 128
        sizes.append(s)
        rem -= s
    assert all(s % 128 == 0 for s in sizes), sizes
    return sizes


def _groups(tc):
    """Split a pass into 512-wide PSUM groups (last may be smaller)."""
    return [(o, min(G, tc - o)) for o in range(0, tc, G)]


def _build_nc(cap):
    import concourse.bass as bass  # noqa: F401
    from concourse import bacc
    import concourse.mybir as mybir
    import concourse.tile as tile

    F32 = mybir.dt.float32
    BF16 = mybir.dt.bfloat16
    Silu = mybir.ActivationFunctionType.Silu
    Mult = mybir.AluOpType.mult

    sizes = _pass_sizes(cap)
    maxtc = max(sizes)

    nc = bacc.Bacc("TRN2", target_bir_lowering=False, debug=False)
    xT = nc.declare_dram_parameter("xT", [D, cap], BF16, isOutput=False)
    wg = nc.declare_dram_parameter("wg", [HT, P, KT * P], BF16, isOutput=False)
    wu = nc.declare_dram_parameter("wu", [HT, P, KT * P], BF16, isOutput=False)
    wd = nc.declare_dram_parameter("wd", [H, D], BF16, isOutput=False)
    wt = nc.declare_dram_parameter("wt", [cap], F32, isOutput=False)
    out = nc.declare_dram_parameter("out", [cap, D], BF16, isOutput=True)

    with tile.TileContext(nc) as tc:
        with (
            tc.tile_pool(name="sbuf", bufs=1) as pool,
            tc.tile_pool(name="psum", bufs=1, space="PSUM") as pp,
        ):
            wd_ts = [None] * HT
            wt_t = None
            t0 = 0
            for pi, TC in enumerate(sizes):
                grps = _groups(TC)
                ng = len(grps)
                # x^T for this pass as one [128, 512] tile per (d-tile,
                # token-block) pair, streamed block-major on the two spare
                # queues (gpsimd + scalar). Fine tiles let the first
                # h-tile's matmuls chase the x stream chunk by chunk.
                xt_bd = [[None] * KT for _ in grps]
                for gi, (off, g) in enumerate(grps):
                    for dti in range(KT):
                        xt_1 = pool.tile([P, G], BF16, tag=f"xt{gi}_{dti}",
                                         bufs=1, name=f"xt{gi}_{dti}")
                        eng = nc.gpsimd if dti % 2 == 0 else nc.scalar
                        eng.dma_start(
                            xt_1[:, :g],
                            xT[dti * P : (dti + 1) * P,
                               t0 + off : t0 + off + g],
                        )
                        xt_bd[gi][dti] = xt_1
                # h^T for this pass
                h_t = pool.tile([P, HT, maxtc], BF16, tag="ht", bufs=1)

                # ---- stage A: h^T = silu(g^T) * u^T ----
                # Wg/Wu stream as [128, 1024] half-tiles so the first
                # matmul only waits for an 8-d-tile half, not a full row
                HW = KT * P // 2
                for ht in range(HT):
                    wg_h = []
                    wu_h = []
                    for hf in range(2):
                        w1 = pool.tile([P, HW], BF16, tag="wgu", bufs=8)
                        nc.sync.dma_start(w1[:], wg[ht, :, hf * HW : (hf + 1) * HW])
                        wg_h.append(w1)
                    for hf in range(2):
                        w1 = pool.tile([P, HW], BF16, tag="wgu", bufs=8)
                        nc.sync.dma_start(w1[:], wu[ht, :, hf * HW : (hf + 1) * HW])
                        wu_h.append(w1)

                    psg = [pp.tile([P, G], F32, tag=f"g{i}", bufs=1,
                                   name=f"psg{i}") for i in range(ng)]
                    psu = [pp.tile([P, G], F32, tag=f"u{i}", bufs=1,
                                   name=f"psu{i}") for i in range(ng)]
                    silu_ts = [None] * ng

                    def a_gate(d, gi, g):
                        lhs = wg_h[d // 8][:, (d % 8) * P : (d % 8 + 1) * P]
                        nc.tensor.matmul(
                            psg[gi][:, :g], lhs, xt_bd[gi][d][:, :g],
                            start=(d == 0), stop=(d == KT - 1),
                        )

                    def a_silu(gi, g):
                        st = pool.tile([P, G], F32, tag=f"silu{gi}", bufs=2,
                                       name="st")
                        nc.scalar.activation(st[:, :g], psg[gi][:, :g], Silu)
                        silu_ts[gi] = st

                    def a_up(d, gi, g):
                        lhs = wu_h[d // 8][:, (d % 8) * P : (d % 8 + 1) * P]
                        nc.tensor.matmul(
                            psu[gi][:, :g], lhs, xt_bd[gi][d][:, :g],
                            start=(d == 0), stop=(d == KT - 1),
                        )

                    def a_mult(gi, off, g):
                        nc.vector.tensor_tensor(
                            h_t[:, ht, off : off + g],
                            silu_ts[gi][:, :g],
                            psu[gi][:, :g],
                            op=Mult,
                        )

                    if ht == 0:
                        # block-major: each 512-token block only needs its
                        # own x chunks, so the PE chases the x stream
                        # instead of stalling on the full 8 MB
                        for gi, (off, g) in enumerate(grps):
                            for d in range(KT):
                                a_gate(d, gi, g)
                            a_silu(gi, g)
                            for d in range(KT):
                                a_up(d, gi, g)
                            a_mult(gi, off, g)
                    else:
                        for d in range(KT):
                            for gi, (off, g) in enumerate(grps):
                                a_gate(d, gi, g)
                        for gi, (off, g) in enumerate(grps):
                            a_silu(gi, g)
                        for d in range(KT):
                            for gi, (off, g) in enumerate(grps):
                                a_up(d, gi, g)
                        for gi, (off, g) in enumerate(grps):
                            a_mult(gi, off, g)

                if pi == 0:
                    # Wd is resident for the whole kernel; stream it on the
                    # sync queue after the Wg/Wu stream so it stays out of
                    # the bandwidth-critical startup window
                    for ht in range(HT):
                        wdc = pool.tile([P, D], BF16, tag=f"wd{ht}", bufs=1,
                                        name=f"wdc{ht}")
                        nc.sync.dma_start(
                            wdc[:], wd[ht * P : (ht + 1) * P, :]
                        )
                        wd_ts[ht] = wdc

                if pi == 0:
                    wt_t = pool.tile([P, cap // P], F32, tag="wt", bufs=1)
                    nc.gpsimd.dma_start(
                        wt_t[:], wt.rearrange("(n p) -> p n", p=P)
                    )

                # ---- stage B: y = h @ Wd^T, scaled by routing weight ----
                for ts_ in range(TC // P):
                    # ping-pong PSUM between the stage-A gate and up banks
                    # so consecutive t-tiles never wait on a WAR hazard
                    pfx = "u" if ts_ % 2 == 0 else "g"
                    psy = [pp.tile([P, G], F32, tag=f"{pfx}{i}",
                                   name=f"psy{i}") for i in range(4)]
                    y_t = pool.tile([P, D], BF16, tag="yout", bufs=2,
                                    name="y_t")
                    col = t0 // P + ts_
                    lhs_base = ts_ * P
                    for dc in range(4):
                        for ht in range(HT):
                            nc.tensor.matmul(
                                psy[dc][:],
                                h_t[:, ht, lhs_base : lhs_base + P],
                                wd_ts[ht][:, dc * G : (dc + 1) * G],
                                start=(ht == 0),
                                stop=(ht == HT - 1),
                            )
                        nc.vector.tensor_scalar_mul(
                            y_t[:, dc * G : (dc + 1) * G],
                            psy[dc][:],
                            wt_t[:, col : col + 1],
                        )
                        nc.scalar.dma_start(
                            out[
                                t0 + ts_ * P : t0 + (ts_ + 1) * P,
                                dc * G : (dc + 1) * G,
                            ],
                            y_t[:, dc * G : (dc + 1) * G],
                        )
                t0 += TC

    nc.finalize()
    return nc


def _get_nc(cap):
    if cap not in _built:
        _built[cap] = _build_nc(cap)
    return _built[cap]


def kernel(x, weights, Wg, Wu, Wd, indices, seq_len=None, **_unused):
    from concourse.bass_utils import run_bass_kernel_spmd
    import ml_dtypes

    bf16 = ml_dtypes.bfloat16

    x = np.asarray(x, dtype=np.float32)
    weights = np.asarray(weights, dtype=np.float32)
    Wg = np.asarray(Wg, dtype=np.float32)
    Wu = np.asarray(Wu, dtype=np.float32)
    Wd = np.asarray(Wd, dtype=np.float32)
    indices = np.asarray(indices).astype(np.int64)

    t, d = x.shape
    e, h, _ = Wg.shape
    k = indices.shape[1]

    # ---- host-side routing (dispatch), merging duplicate (token, expert)
    # slots so their routing weights add and each pair is computed once ----
    flat_e = indices.reshape(-1)
    flat_t = np.repeat(np.arange(t, dtype=np.int64), k)
    flat_w = weights.reshape(-1)
    key = flat_e * t + flat_t                       # sort by (expert, token)
    order = np.argsort(key, kind="stable")
    ks = key[order]
    ws = flat_w[order]
    uniq = np.empty(len(ks), dtype=bool)
    uniq[0] = True
    uniq[1:] = ks[1:] != ks[:-1]
    seg = np.cumsum(uniq) - 1
    w_u = np.bincount(seg, weights=ws).astype(np.float32)  # summed weights
    k_u = ks[uniq]
    e_u = k_u // t
    tok_u = k_u % t
    counts = np.bincount(e_u, minlength=e)
    starts = np.zeros(e + 1, dtype=np.int64)
    starts[1:] = np.cumsum(counts)
    cap = int(-(-max(int(counts.max()), 512) // P) * P)

    in_maps = []
    for ei in range(e):
        n = int(counts[ei])
        toks = tok_u[starts[ei] : starts[ei] + n]
        xe = np.zeros((cap, d), dtype=np.float32)
        xe[:n] = x[toks]
        wvec = np.zeros(cap, dtype=np.float32)
        wvec[:n] = w_u[starts[ei] : starts[ei] + n]
        # pack Wg/Wu so each h-tile block is one contiguous [128, 2048] DMA:
        # block[ht][p][k*128+hh] = Wg[e].T[k*128+p, ht*128+hh]
        WgT = Wg[ei].T  # [D, H]
        WuT = Wu[ei].T
        wg_lin = np.ascontiguousarray(
            WgT.reshape(KT, P, HT, P).transpose(2, 1, 0, 3).reshape(HT, P, KT * P)
        ).astype(bf16)
        wu_lin = np.ascontiguousarray(
            WuT.reshape(KT, P, HT, P).transpose(2, 1, 0, 3).reshape(HT, P, KT * P)
        ).astype(bf16)
        wdT = np.ascontiguousarray(Wd[ei].T).astype(bf16)  # [H, D]
        in_maps.append(
            {
                "xT": np.ascontiguousarray(xe.T).astype(bf16),
                "wg": wg_lin,
                "wu": wu_lin,
                "wd": wdT,
                "wt": wvec,
            }
        )

    nc = _get_nc(cap)
    trace = bool(int(os.environ.get("KERNEL_TRACE", "0")))
    res = run_bass_kernel_spmd(
        nc, in_maps, core_ids=list(range(e)), trace=trace
    )
    if trace:
        kernel.last_exec_time_ns = res.exec_time_ns
        kernel.last_results = res

    # ---- host-side combine ----
    allres = np.concatenate(
        [np.asarray(res.results[ei]["out"][: counts[ei]]) for ei in range(e)],
        axis=0,
    ).astype(np.float32)
    allres = np.vstack([allres, np.zeros((1, d), np.float32)])
    # map each original (token, k) slot to its deduped row; the second slot
    # of an intra-token duplicate points at the trailing zeros row
    pos = np.searchsorted(k_u, key).reshape(t, k)
    dup = indices[:, 0] == indices[:, 1]
    pos[dup, 1] = len(k_u)
    y = allres[pos[:, 0]] + allres[pos[:, 1]]
    return y


# revision 12
# speedup vs baseline: 1.2024x; 1.0327x over previous
"""MoE expert-MLP (SwiGLU) kernel for 8 Trainium2 NeuronCores.

Strategy: expert-parallel, one expert per core. Host-side routing dedups
duplicate (token, expert) slots (K=2 can pick the same expert twice; the
routing weights then just add), which drops the per-expert max count under
2048 and saves a full 128-token tile of padding. Each core runs a dense
[cap, D] SwiGLU MLP in bf16 (tolerance is 2e-2; bf16 lands ~5e-3) which
halves DMA traffic and SBUF footprint vs fp32r at the same PE rate.

Per-core kernel runs block-major over 512-token blocks, with stages
interleaved per block so HBM demand stays flat (~350 GB/s would otherwise
be needed in a front-loaded stage A):
  stage A(b): h^T[h, b] = silu(Wg @ x^T[b]) * (Wu @ x^T[b])
  stage B(b): y[b, d]  = (h^T[b])^T @ Wd^T, row-scaled by routing weight
Wg/Wu/Wd stay SBUF-resident (bf16), x streams one block ahead (the first
block as 16 fine tiles the PE chases chunk-by-chunk at startup, later
blocks as single 2 MB strided DMAs), h lives one block at a time. PSUM is
8 uniform [128, 512] f32 banks: stage A rotates gate/up accumulators by
h-tile parity (4 banks), stage B ping-pongs its 4 output accumulators
between its own bank set and stage A's by t-tile parity, so nothing ever
stalls on a write-after-read hazard.
"""

import sys
import os

sys.path.insert(0, "/opt/trn_rl_repo")

import numpy as np

T, D, H, E, K = 8192, 2048, 1408, 8, 2
P = 128
HT = H // P        # 11 h-tiles
KT = D // P        # 16 d-tiles
G = 512            # PSUM group width (one bank of f32) = token block

_built = {}


def _pass_sizes(cap):
    """Split cap into passes of <=2048 tokens (multiples of 128).
    Expected input fits in a single pass of 2048."""
    sizes = []
    rem = cap
    while rem:
        s = min(rem, 2048)
        if rem - s == 128:  # avoid a tiny trailing pass
            s -= 128
        sizes.append(s)
        rem -= s
    assert all(s % 128 == 0 for s in sizes), sizes
    return sizes


def _groups(tc):
    """Split a pass into 512-wide token blocks (last may be smaller)."""
    return [(o, min(G, tc - o)) for o in range(0, tc, G)]


def _build_nc(cap):
    import concourse.bass as bass  # noqa: F401
    from concourse import bacc
    import concourse.mybir as mybir
    import concourse.tile as tile

    F32 = mybir.dt.float32
    BF16 = mybir.dt.bfloat16
    Silu = mybir.ActivationFunctionType.Silu
    Mult = mybir.AluOpType.mult

    sizes = _pass_sizes(cap)

    nc = bacc.Bacc("TRN2", target_bir_lowering=False, debug=False)
    xT = nc.declare_dram_parameter("xT", [D, cap], BF16, isOutput=False)
    wg = nc.declare_dram_parameter("wg", [HT, P, KT * P], BF16, isOutput=False)
    wu = nc.declare_dram_parameter("wu", [HT, P, KT * P], BF16, isOutput=False)
    wd = nc.declare_dram_parameter("wd", [H, D], BF16, isOutput=False)
    wt = nc.declare_dram_parameter("wt", [cap], F32, isOutput=False)
    out = nc.declare_dram_parameter("out", [cap, D], BF16, isOutput=True)

    xTr = xT.rearrange("(k p) t -> p k t", p=P)  # [128, KT, cap] view
    HW = KT * P // 2

    with tile.TileContext(nc) as tc:
        with (
            tc.tile_pool(name="sbuf", bufs=1) as pool,
            tc.tile_pool(name="psum", bufs=1, space="PSUM") as pp,
        ):
            # ---- resident weights: Wg/Wu as per-h-tile halves ----
            wg_h = []
            wu_h = []
            for ht in range(HT):
                gh, uh = [], []
                for hf in range(2):
                    w1 = pool.tile([P, HW], BF16, tag=f"wg{ht}h{hf}", bufs=1)
                    nc.sync.dma_start(w1[:], wg[ht, :, hf * HW : (hf + 1) * HW])
                    gh.append(w1)
                for hf in range(2):
                    w1 = pool.tile([P, HW], BF16, tag=f"wu{ht}h{hf}", bufs=1)
                    nc.sync.dma_start(w1[:], wu[ht, :, hf * HW : (hf + 1) * HW])
                    uh.append(w1)
                wg_h.append(gh)
                wu_h.append(uh)
            # Wd resident, streamed after the Wg/Wu stream (needed ~80us in)
            wd_ts = []
            for ht in range(HT):
                wdc = pool.tile([P, D], BF16, tag=f"wd{ht}", bufs=1,
                                name=f"wdc{ht}")
                nc.sync.dma_start(wdc[:], wd[ht * P : (ht + 1) * P, :])
                wd_ts.append(wdc)
            # routing weights, one f32 column per t-tile
            wt_t = pool.tile([P, cap // P], F32, tag="wt", bufs=1)
            nc.gpsimd.dma_start(wt_t[:], wt.rearrange("(n p) -> p n", p=P))

            first_block = True
            t0 = 0
            for pi, TC in enumerate(sizes):
                grps = _groups(TC)

                # x for block 0 of the first pass: 16 fine [128, 512] tiles
                # on two queues, so the PE starts ~1us in and chases the
                # stream chunk by chunk; later blocks: one strided DMA
                # prefetched a block ahead into 2 rotating big tiles.
                xblk = [None] * len(grps)

                def load_fine(off, g):
                    ts_ = []
                    for d in range(KT):
                        x1 = pool.tile([P, G], BF16, tag=f"xf{d}", bufs=1,
                                       name=f"xf{d}")
                        eng = nc.gpsimd if d % 2 == 0 else nc.scalar
                        eng.dma_start(
                            x1[:, :g],
                            xT[d * P : (d + 1) * P, t0 + off : t0 + off + g],
                        )
                        ts_.append(x1)
                    return lambda d: ts_[d]

                def load_big(off, g):
                    x1 = pool.tile([P, KT, G], BF16, tag="xb", bufs=2,
                                   name="xb")
                    nc.gpsimd.dma_start(
                        x1[:, :, :g], xTr[:, :, t0 + off : t0 + off + g]
                    )
                    return lambda d: x1[:, d, :]

                for bi, (off, g) in enumerate(grps):
                    if first_block:
                        xblk[0] = load_fine(off, g)
                        first_block = False
                        if len(grps) > 1:
                            xblk[1] = load_big(*grps[1])
                    if xblk[bi] is None:
                        xblk[bi] = load_big(off, g)
                    xs = xblk[bi]

                    # prefetch next block's x one block ahead
                    nb = bi + 1
                    if nb < len(grps) and xblk[nb] is None:
                        xblk[nb] = load_big(*grps[nb])

                    # h^T for this block only
                    h_t = pool.tile([P, HT, G], BF16, tag="ht", bufs=1)

                    # ---- stage A(b) ----
                    for ht in range(HT):
                        psg = pp.tile([P, G], F32, tag=f"g{ht % 2}",
                                      name="psg")
                        psu = pp.tile([P, G], F32, tag=f"u{ht % 2}",
                                      name="psu")
                        for d in range(KT):
                            lhs = wg_h[ht][d // 8][:, (d % 8) * P : (d % 8 + 1) * P]
                            nc.tensor.matmul(
                                psg[:, :g], lhs, xs(d)[:, :g],
                                start=(d == 0), stop=(d == KT - 1),
                            )
                        st = pool.tile([P, G], F32, tag="silu", bufs=2,
                                       name="st")
                        nc.scalar.activation(st[:, :g], psg[:, :g], Silu)
                        for d in range(KT):
                            lhs = wu_h[ht][d // 8][:, (d % 8) * P : (d % 8 + 1) * P]
                            nc.tensor.matmul(
                                psu[:, :g], lhs, xs(d)[:, :g],
                                start=(d == 0), stop=(d == KT - 1),
                            )
                        nc.vector.tensor_tensor(
                            h_t[:, ht, :g], st[:, :g], psu[:, :g], op=Mult,
                        )

                    # ---- stage B(b): 4 t-tiles of 128 tokens ----
                    for ts_ in range(g // P):
                        ptags = (["y0", "y1", "y2", "y3"] if ts_ % 2 == 0
                                 else ["g0", "g1", "u0", "u1"])
                        psy = [pp.tile([P, G], F32, tag=ptags[i],
                                       name=f"psy{i}") for i in range(4)]
                        y_t = pool.tile([P, D], BF16, tag="yout", bufs=2,
                                        name="y_t")
                        col = (t0 + off) // P + ts_
                        for dc in range(4):
                            for ht in range(HT):
                                nc.tensor.matmul(
                                    psy[dc][:],
                                    h_t[:, ht, ts_ * P : (ts_ + 1) * P],
                                    wd_ts[ht][:, dc * G : (dc + 1) * G],
                                    start=(ht == 0),
                                    stop=(ht == HT - 1),
                                )
                            nc.vector.tensor_scalar_mul(
                                y_t[:, dc * G : (dc + 1) * G],
                                psy[dc][:],
                                wt_t[:, col : col + 1],
                            )
                        nc.sync.dma_start(
                            out[t0 + off + ts_ * P : t0 + off + (ts_ + 1) * P, :],
                            y_t[:],
                        )
                t0 += TC

    nc.finalize()
    return nc


def _get_nc(cap):
    if cap not in _built:
        _built[cap] = _build_nc(cap)
    return _built[cap]


def kernel(x, weights, Wg, Wu, Wd, indices, seq_len=None, **_unused):
    from concourse.bass_utils import run_bass_kernel_spmd
    import ml_dtypes

    bf16 = ml_dtypes.bfloat16

    x = np.asarray(x, dtype=np.float32)
    weights = np.asarray(weights, dtype=np.float32)
    Wg = np.asarray(Wg, dtype=np.float32)
    Wu = np.asarray(Wu, dtype=np.float32)
    Wd = np.asarray(Wd, dtype=np.float32)
    indices = np.asarray(indices).astype(np.int64)

    t, d = x.shape
    e, h, _ = Wg.shape
    k = indices.shape[1]

    # ---- host-side routing (dispatch), merging duplicate (token, expert)
    # slots so their routing weights add and each pair is computed once ----
    flat_e = indices.reshape(-1)
    flat_t = np.repeat(np.arange(t, dtype=np.int64), k)
    flat_w = weights.reshape(-1)
    key = flat_e * t + flat_t                       # sort by (expert, token)
    order = np.argsort(key, kind="stable")
    ks = key[order]
    ws = flat_w[order]
    uniq = np.empty(len(ks), dtype=bool)
    uniq[0] = True
    uniq[1:] = ks[1:] != ks[:-1]
    seg = np.cumsum(uniq) - 1
    w_u = np.bincount(seg, weights=ws).astype(np.float32)  # summed weights
    k_u = ks[uniq]
    e_u = k_u // t
    tok_u = k_u % t
    counts = np.bincount(e_u, minlength=e)
    starts = np.zeros(e + 1, dtype=np.int64)
    starts[1:] = np.cumsum(counts)
    cap = int(-(-max(int(counts.max()), 512) // P) * P)

    in_maps = []
    for ei in range(e):
        n = int(counts[ei])
        toks = tok_u[starts[ei] : starts[ei] + n]
        xe = np.zeros((cap, d), dtype=np.float32)
        xe[:n] = x[toks]
        wvec = np.zeros(cap, dtype=np.float32)
        wvec[:n] = w_u[starts[ei] : starts[ei] + n]
        # pack Wg/Wu so each h-tile block is one contiguous [128, 2048] DMA:
        # block[ht][p][k*128+hh] = Wg[e].T[k*128+p, ht*128+hh]
        WgT = Wg[ei].T  # [D, H]
        WuT = Wu[ei].T
        wg_lin = np.ascontiguousarray(
            WgT.reshape(KT, P, HT, P).transpose(2, 1, 0, 3).reshape(HT, P, KT * P)
        ).astype(bf16)
        wu_lin = np.ascontiguousarray(
            WuT.reshape(KT, P, HT, P).transpose(2, 1, 0, 3).reshape(HT, P, KT * P)
        ).astype(bf16)
        wdT = np.ascontiguousarray(Wd[ei].T).astype(bf16)  # [H, D]
        in_maps.append(
            {
                "xT": np.ascontiguousarray(xe.T).astype(bf16),
                "wg": wg_lin,
                "wu": wu_lin,
                "wd": wdT,
                "wt": wvec,
            }
        )

    nc = _get_nc(cap)
    trace = bool(int(os.environ.get("KERNEL_TRACE", "0")))
    res = run_bass_kernel_spmd(
        nc, in_maps, core_ids=list(range(e)), trace=trace
    )
    if trace:
        kernel.last_exec_time_ns = res.exec_time_ns
        kernel.last_results = res

    # ---- host-side combine ----
    allres = np.concatenate(
        [np.asarray(res.results[ei]["out"][: counts[ei]]) for ei in range(e)],
        axis=0,
    ).astype(np.float32)
    allres = np.vstack([allres, np.zeros((1, d), np.float32)])
    # map each original (token, k) slot to its deduped row; the second slot
    # of an intra-token duplicate points at the trailing zeros row
    pos = np.searchsorted(k_u, key).reshape(t, k)
    dup = indices[:, 0] == indices[:, 1]
    pos[dup, 1] = len(k_u)
    y = allres[pos[:, 0]] + allres[pos[:, 1]]
    return y
